# revision 16
# baseline (speedup 1.0000x reference)
"""GAT message-passing kernel for Trainium2 (8 NeuronCores, SPMD).

Problem (per full input):
    B=8, S=512, N=32 neighbors, H=256, V=100001
    out[b,s,:] = sum_n softmax_n(leakyrelu(a_w . [src, cand_n]) + mask*NEG) * cand_n
    candidates = [self] + 32 neighbors (self never masked)

Sharding: data-parallel over B — core c handles batch row c.

Design (what matters for speed on TRN2):
  - SWDGE has ~1us fixed cost per DMA instruction; per-slot indirect
    gathers are descriptor-generation bound. Instead the kernel uses
    InstDMAGatherAnt (gpsimd.dma_gather): ONE instruction carries up to
    1024 row-indices (~the HW ring limit), so ~10 gather instructions
    move all candidate rows per core.
  - dma_gather indices are int16, so the host builds a PER-CORE compact
    table holding only the core's ~9.3k unique candidate ids (index
    remap, like the candidate compaction itself).
  - Rows are bf16 with the attention linear folded in at 768B stride
    (%256 required): [emb(256) | zc=emb.awc+b | zs=emb.aws | pad], so
    per-pair logits are a lookup (z = zs[src] + zc[cand]), never a
    256-wide on-chip dot (DVE measures ~420ns/op for those).
  - Masked/pad slots point at a sentinel row (emb=0, zc=-1e5) so their
    softmax weight underflows to exactly 0 and no mask tensor is needed.
  - Weighted aggregation sum_n e_n*F_n accumulates on TensorE in PSUM:
    slots alternate between diag(e_n)@F_n (diag built on DVE) and
    ident@(e_n*F_n) (scaled rhs built on Act) to balance engine load;
    1/sum(e) folds into the PSUM-evacuation scale.
"""

import numpy as np
import ml_dtypes

B, S, N, H, V = 8, 512, 32, 256, 100001
P = 128
S_TILES = S // P
SLOPE = 0.2
N_CORES = 8
HA = 384  # stored row: emb(256) + zc + zs + pad  (768B, %256)
ZC_COL = H
ZS_COL = H + 1
SENT = V  # sentinel id for pad slots (remapped per core)
ZSENT = -1.0e5

# Tuning knobs
CHUNK_IDXS = 512    # row-indices per dma_gather instruction
N_QUEUES = 4        # SWDGE queues (descriptor gen parallelizes across them)
DG_DVE_MOD = 10     # slots n with n % MOD < TAKE weight via DVE-diag path,
DG_DVE_TAKE = 6     # ... the rest via Act scaled-rhs path

BLOCK = 8  # slots per stream block (1024 idxs at P=128)

_CACHE: dict = {}


def _stream_blocks(ncc_list):
    """Round-robin blocks of BLOCK slots across s-tiles; returns
    [(tile, lo, hi, gpos)] where gpos is the block's global stream slot."""
    out = []
    pos = 0
    b = 0
    while True:
        any_blk = False
        for t in range(S_TILES):
            lo = b * BLOCK
            if lo >= ncc_list[t]:
                continue
            hi = min(lo + BLOCK, ncc_list[t])
            out.append((t, lo, hi, pos))
            pos += hi - lo
            any_blk = True
        if not any_blk:
            return out
        b += 1


def _build_nc(cfg):
    ncc_list, u_pad = cfg
    import concourse.bacc as bacc
    import concourse.mybir as mybir
    import concourse.tile as tile
    from concourse.library_config import mlp

    f32 = mybir.dt.float32
    bf16 = mybir.dt.bfloat16
    i16 = mybir.dt.int16
    Alu = mybir.AluOpType
    Act = mybir.ActivationFunctionType
    X = mybir.AxisListType.X

    tot = sum(ncc_list)
    n_idx = tot * P
    chunks = []  # (idx_start, n) in global slot space
    s = 0
    while s < n_idx:
        n = min(CHUNK_IDXS, n_idx - s)
        chunks.append((s, n))
        s += n
    icols = sum(n // 16 for _, n in chunks)
    blocks = _stream_blocks(ncc_list)  # [(tile, lo, hi, gpos)]

    nc = bacc.Bacc(
        "TRN2",
        target_bir_lowering=False,
        debug=False,
        enable_asserts=False,
        num_devices=N_CORES,
        num_swdge_queues=N_QUEUES,
    )

    tab_d = nc.dram_tensor("tab", [u_pad, HA], bf16, kind="ExternalInput").ap()
    idx_d = nc.dram_tensor("idxs", [P, icols], i16, kind="ExternalInput").ap()
    id_d = nc.dram_tensor("identity", [P, P], bf16, kind="ExternalInput").ap()
    out_d = nc.dram_tensor("out", [S, H], f32, kind="ExternalOutput").ap()

    with tile.TileContext(nc) as tc:
        with (
            tc.tile_pool(name="cpool", bufs=1) as cpool,
            tc.tile_pool(name="spool", bufs=5) as spool,
            tc.tile_pool(name="dpool", bufs=16) as dpool,
            tc.tile_pool(name="wpool", bufs=12) as wpool,
            tc.tile_pool(name="ppool", bufs=4, space="PSUM") as ppool,
        ):
            nc.gpsimd.load_library(mlp)
            idx = cpool.tile([P, icols], i16)
            nc.sync.dma_start(out=idx[:], in_=idx_d)

            F = cpool.tile([P, tot * HA], bf16)
            F3 = F.rearrange("p (n h) -> p n h", n=tot)

            ident = cpool.tile([P, P], bf16)
            nc.sync.dma_start(out=ident[:], in_=id_d)

            nregs = {n: nc.gpsimd.to_reg(n) for n in sorted({n for _, n in chunks})}
            col = 0
            for ci, (s0, n) in enumerate(chunks):
                k = n // P  # slots covered
                g0 = s0 // P
                nc.gpsimd.dma_gather(
                    F3[:, g0 : g0 + k, :],
                    tab_d,
                    idx[:, col : col + n // 16],
                    n,
                    nregs[n],
                    HA,
                    queue_num=ci % N_QUEUES,
                )
                col += n // 16

            # chunk boundaries in global stream-slot space
            cb = sorted({s0 // P for s0, _ in chunks} | {tot})

            zsrc_t, zl_t, z2_t, e_t, deng_t, acc_t, gi_t = {}, {}, {}, {}, {}, {}, {}
            ngroups = [0] * S_TILES
            for (t, lo, hi, gpos) in blocks:
                splits = sorted({gpos, gpos + (hi - lo)} | {x for x in cb if gpos < x < gpos + (hi - lo)})
                ngroups[t] += len(splits) - 1

            for (t, lo, hi, gpos) in blocks:
                ncc = ncc_list[t]
                if lo == 0:
                    zsrc = spool.tile([P, 1], f32, name="zsrc")
                    nc.vector.tensor_copy(zsrc[:], F3[:, gpos, ZS_COL].unsqueeze(1))
                    zsrc_t[t] = zsrc
                    zl_t[t] = spool.tile([P, ncc], f32, name="zl")
                    z2_t[t] = spool.tile([P, ncc], f32, name="z2")
                    e_t[t] = spool.tile([P, ncc], f32, name="e")
                    deng_t[t] = spool.tile([P, ngroups[t]], f32, name="deng")
                    acc_t[t] = ppool.tile([P, H], f32, name="acc")
                    gi_t[t] = 0
                zsrc, zl, z2, e = zsrc_t[t], zl_t[t], z2_t[t], e_t[t]
                deng, acc = deng_t[t], acc_t[t]
                splits = sorted({gpos, gpos + (hi - lo)} | {x for x in cb if gpos < x < gpos + (hi - lo)})
                for ga, gb in zip(splits[:-1], splits[1:]):
                    la = lo + (ga - gpos)
                    lb = la + (gb - ga)
                    gi = gi_t[t]
                    gi_t[t] += 1
                    nc.vector.tensor_scalar_add(
                        z2[:, la:lb], F3[:, ga:gb, ZC_COL], zsrc[:]
                    )
                    nc.vector.scalar_tensor_tensor(
                        out=zl[:, la:lb],
                        in0=z2[:, la:lb],
                        scalar=SLOPE,
                        in1=z2[:, la:lb],
                        op0=Alu.mult,
                        op1=Alu.max,
                    )
                    nc.scalar.activation(
                        e[:, la:lb],
                        zl[:, la:lb],
                        Act.Exp,
                        accum_out=deng[:, gi : gi + 1],
                    )
                    for g in range(ga, gb):
                        ln = la + (g - ga)
                        if ln % DG_DVE_MOD < DG_DVE_TAKE:
                            dg = dpool.tile([P, P], bf16, name="dg")
                            nc.vector.tensor_scalar_mul(
                                dg[:], ident[:], e[:, ln : ln + 1]
                            )
                            lhsT, rhs = dg[:], F3[:, g, 0:H]
                        else:
                            fw = wpool.tile([P, H], bf16, name="fw")
                            nc.scalar.mul(fw[:], F3[:, g, 0:H], e[:, ln : ln + 1])
                            lhsT, rhs = ident[:], fw[:]
                        nc.tensor.matmul(
                            out=acc[:],
                            lhsT=lhsT,
                            rhs=rhs,
                            start=(ln == 0),
                            stop=(ln == ncc - 1),
                        )
                if hi == ncc:
                    rows = slice(t * P, (t + 1) * P)
                    den = spool.tile([P, 1], f32, name="den")
                    nc.vector.tensor_reduce(den[:], deng[:], axis=X, op=Alu.add)
                    rden = spool.tile([P, 1], f32, name="rden")
                    nc.vector.reciprocal(rden[:], den[:])
                    o = spool.tile([P, H], f32, name="o")
                    nc.scalar.mul(o[:], acc[:], rden[:])
                    nc.sync.dma_start(out=out_d[rows, :], in_=o[:])

    nc.compile()
    return nc


def _get_nc(ncc_list, u_pad):
    key = (tuple(ncc_list), u_pad)
    if key not in _CACHE:
        _CACHE[key] = _build_nc(key)
    return _CACHE[key]


def _ensure_axon_hooks():
    """Provide antenv.axon_hooks if the image lacks it, so trace=True /
    BASS_TRACE=1 profiling requests don't crash run_bass_kernel_spmd."""
    import sys
    import types

    try:
        import antenv.axon_hooks  # noqa: F401

        return
    except ImportError:
        pass
    try:
        import antenv
    except ImportError:
        return
    mod = types.ModuleType("antenv.axon_hooks")
    state = {"hook": None}

    def set_axon_ntff_profile_hook(h):
        state["hook"] = h

    def get_axon_ntff_profile_hook():
        if state["hook"] is None:
            try:
                from trn_agent_boot.trn_boot import _ntff_profile_via_ctypes

                state["hook"] = _ntff_profile_via_ctypes("/opt/axon/libaxon_pjrt.so")
            except Exception:
                return None
        return state["hook"]

    mod.set_axon_ntff_profile_hook = set_axon_ntff_profile_hook
    mod.get_axon_ntff_profile_hook = get_axon_ntff_profile_hook
    sys.modules["antenv.axon_hooks"] = mod
    antenv.axon_hooks = mod


def _prepare(inputs):
    """Host-side prep: per-core compact bf16 tables + slot-major int16
    index streams in the dma_gather wrapped layout."""
    node_ids = np.asarray(inputs["node_ids"]).astype(np.int64).reshape(B, S)
    neighs = np.asarray(inputs["neighs"]).astype(np.int64).reshape(B, S, N)
    mask = np.asarray(inputs["mask"]).astype(np.int32).reshape(B, S, N)
    emb = np.ascontiguousarray(np.asarray(inputs["emb_table"], dtype=np.float32))
    a_w = np.asarray(inputs["a_w"], dtype=np.float32).reshape(2 * H, 1)
    a_b = np.asarray(inputs["a_b"], dtype=np.float32)

    aws = a_w[:H, 0]
    awc = a_w[H:, 0]
    ab = np.float32(a_b.reshape(-1)[0])

    # Compact candidates: unmasked neighbors first, self at slot 0, pads
    # point at the sentinel. Sort nodes by unmasked count (desc) so later
    # tiles need fewer slots.
    un_cnt = (mask == 0).sum(axis=-1)  # [B, S]
    perm = np.argsort(-un_cnt, axis=1, kind="stable")
    nid_p = np.take_along_axis(node_ids, perm, axis=1)
    nbr_p = np.take_along_axis(neighs, perm[..., None], axis=1)
    msk_p = np.take_along_axis(mask, perm[..., None], axis=1)
    cnt_p = np.take_along_axis(un_cnt, perm, axis=1)

    cnt_t = cnt_p.reshape(B, S_TILES, P)
    ncc_list = [max(int(cnt_t[:, t, :].max()) + 1, 2) for t in range(S_TILES)]
    ncc = max(ncc_list)

    order = np.argsort(msk_p, axis=-1, kind="stable")
    sneighs = np.take_along_axis(nbr_p, order, axis=-1)
    cands = np.empty((B, S, ncc), np.int64)
    cands[..., 0] = nid_p
    cands[..., 1:] = sneighs[..., : ncc - 1]
    ks = np.arange(1, ncc)[None, None, :]
    cands[..., 1:][ks > cnt_p[..., None]] = SENT

    # Per-core compact tables and local-id index streams
    tabs, idx_streams, u_list = [], [], []
    for c in range(N_CORES):
        uniq, inv = np.unique(cands[c], return_inverse=True)
        u = len(uniq)
        assert u <= 32000, u
        loc = inv.reshape(S, ncc).astype(np.int16)
        tab = np.zeros((u, HA), dtype=ml_dtypes.bfloat16)
        real = uniq != SENT
        rows = emb[uniq[real]]
        tab[real, 0:H] = rows.astype(ml_dtypes.bfloat16)
        zc = rows @ awc + ab
        zs = rows @ aws
        np.clip(zc, -30.0, 30.0, out=zc)
        np.clip(zs, -30.0, 30.0, out=zs)
        tab[real, ZC_COL] = zc.astype(ml_dtypes.bfloat16)
        tab[real, ZS_COL] = zs.astype(ml_dtypes.bfloat16)
        tab[~real, ZC_COL] = np.float32(ZSENT)
        tabs.append(tab)
        u_list.append(u)

        # tile-interleaved, slot-major index stream
        stream = []
        for (t, lo, hi, gpos) in _stream_blocks(ncc_list):
            blk = loc[t * P : (t + 1) * P, lo:hi]  # [P, hi-lo]
            stream.append(blk.T.reshape(-1))  # slot-major
        idx_streams.append(np.concatenate(stream))

    u_pad = max(u_list)
    tabs = [
        np.ascontiguousarray(np.vstack([t, np.zeros((u_pad - len(t), HA), t.dtype)]))
        if len(t) < u_pad
        else np.ascontiguousarray(t)
        for t in tabs
    ]

    # wrapped idx layout per chunk: idx i -> partition i%16, col i//16,
    # replicated across the 8 Q7-core stripes
    n_idx = sum(ncc_list) * P
    idxw_all = []
    for c in range(N_CORES):
        st = idx_streams[c]
        assert len(st) == n_idx
        cols = []
        s = 0
        while s < n_idx:
            n = min(CHUNK_IDXS, n_idx - s)
            blk = st[s : s + n].reshape(n // 16, 16).T
            cols.append(np.tile(blk, (8, 1)))
            s += n
        idxw_all.append(np.ascontiguousarray(np.hstack(cols).astype(np.int16)))

    return tabs, idxw_all, perm, ncc_list, u_pad


def kernel(**inputs) -> np.ndarray:
    _ensure_axon_hooks()
    from concourse.bass_utils import run_bass_kernel_spmd

    tabs, idxw_all, perm, ncc_list, u_pad = _prepare(inputs)
    nc = _get_nc(ncc_list, u_pad)
    identity = np.ascontiguousarray(np.eye(P, dtype=ml_dtypes.bfloat16))
    in_maps = [
        {"tab": tabs[c], "idxs": idxw_all[c], "identity": identity}
        for c in range(N_CORES)
    ]
    core_ids = list(range(N_CORES))
    try:
        res = run_bass_kernel_spmd(nc, in_maps, core_ids=core_ids)
    except Exception:
        # transient device wedge — retry once
        res = run_bass_kernel_spmd(nc, in_maps, core_ids=core_ids)
    _CACHE["last_res"] = res
    out = np.empty((N_CORES, S, H), np.float32)
    for c in range(N_CORES):
        out[c, perm[c], :] = res.results[c]["out"]
    return out


# revision 17
# speedup vs baseline: 1.1017x; 1.1017x over previous
"""GAT message-passing kernel for Trainium2 (8 NeuronCores, SPMD).

Problem (per full input):
    B=8, S=512, N=32 neighbors, H=256, V=100001
    out[b,s,:] = sum_n softmax_n(leakyrelu(a_w . [src, cand_n]) + mask*NEG) * cand_n
    candidates = [self] + 32 neighbors (self never masked)

Sharding: data-parallel over B — core c handles batch row c.

Design (what matters for speed on TRN2):
  - SWDGE has ~1us fixed cost per DMA instruction; per-slot indirect
    gathers are descriptor-generation bound. Instead the kernel uses
    InstDMAGatherAnt (gpsimd.dma_gather): ONE instruction carries up to
    1024 row-indices (~the HW ring limit), so ~10 gather instructions
    move all candidate rows per core.
  - dma_gather indices are int16, so the host builds a PER-CORE compact
    table holding only the core's ~9.3k unique candidate ids (index
    remap, like the candidate compaction itself).
  - Rows are bf16 with the attention linear folded in at 768B stride
    (%256 required): [emb(256) | zc=emb.awc+b | zs=emb.aws | pad], so
    per-pair logits are a lookup (z = zs[src] + zc[cand]), never a
    256-wide on-chip dot (DVE measures ~420ns/op for those).
  - Masked/pad slots point at a sentinel row (emb=0, zc=-1e5) so their
    softmax weight underflows to exactly 0 and no mask tensor is needed.
  - Weighted aggregation sum_n e_n*F_n accumulates on TensorE in PSUM:
    slots alternate between diag(e_n)@F_n (diag built on DVE) and
    ident@(e_n*F_n) (scaled rhs built on Act) to balance engine load;
    1/sum(e) folds into the PSUM-evacuation scale.
"""

import numpy as np
import ml_dtypes

B, S, N, H, V = 8, 512, 32, 256, 100001
P = 128
S_TILES = S // P
SLOPE = 0.2
N_CORES = 8
HA = 384  # stored row: emb(256) + zc + zs + pad  (768B, %256)
ZC_COL = H
ZS_COL = H + 1
SENT = V  # sentinel id for pad slots (remapped per core)
ZSENT = -1.0e5

# Tuning knobs
CHUNK_IDXS = 1024   # row-indices per dma_gather instruction (HW ring limit)
N_QUEUES = 4        # SWDGE queues (descriptor gen parallelizes across them)
DG_DVE_MOD = 10     # slots n with n % MOD < TAKE weight via DVE-diag path,
DG_DVE_TAKE = 6     # ... the rest via Act scaled-rhs path

BLOCK = 8  # slots per stream block (1024 idxs at P=128)

_CACHE: dict = {}


def _stream_blocks(ncc_list):
    """Round-robin blocks of BLOCK slots across s-tiles; returns
    [(tile, lo, hi, gpos)] where gpos is the block's global stream slot."""
    out = []
    pos = 0
    b = 0
    while True:
        any_blk = False
        for t in range(S_TILES):
            lo = b * BLOCK
            if lo >= ncc_list[t]:
                continue
            hi = min(lo + BLOCK, ncc_list[t])
            out.append((t, lo, hi, pos))
            pos += hi - lo
            any_blk = True
        if not any_blk:
            return out
        b += 1


def _build_nc(cfg):
    ncc_list, u_pad = cfg
    import concourse.bacc as bacc
    import concourse.mybir as mybir
    import concourse.tile as tile
    from concourse.library_config import mlp

    f32 = mybir.dt.float32
    bf16 = mybir.dt.bfloat16
    i16 = mybir.dt.int16
    Alu = mybir.AluOpType
    Act = mybir.ActivationFunctionType
    X = mybir.AxisListType.X

    tot = sum(ncc_list)
    n_idx = tot * P
    chunks = []  # (idx_start, n) in global slot space
    s = 0
    while s < n_idx:
        n = min(CHUNK_IDXS, n_idx - s)
        chunks.append((s, n))
        s += n
    icols = sum(n // 16 for _, n in chunks)
    blocks = _stream_blocks(ncc_list)  # [(tile, lo, hi, gpos)]

    nc = bacc.Bacc(
        "TRN2",
        target_bir_lowering=False,
        debug=False,
        enable_asserts=False,
        num_devices=N_CORES,
        num_swdge_queues=N_QUEUES,
    )

    tab_d = nc.dram_tensor("tab", [u_pad, HA], bf16, kind="ExternalInput").ap()
    idx_d = nc.dram_tensor("idxs", [P, icols], i16, kind="ExternalInput").ap()
    id_d = nc.dram_tensor("identity", [P, P], bf16, kind="ExternalInput").ap()
    out_d = nc.dram_tensor("out", [S, H], f32, kind="ExternalOutput").ap()

    with tile.TileContext(nc) as tc:
        with (
            tc.tile_pool(name="cpool", bufs=1) as cpool,
            tc.tile_pool(name="spool", bufs=5) as spool,
            tc.tile_pool(name="dpool", bufs=16) as dpool,
            tc.tile_pool(name="wpool", bufs=12) as wpool,
            tc.tile_pool(name="ppool", bufs=4, space="PSUM") as ppool,
        ):
            nc.gpsimd.load_library(mlp)
            idx = cpool.tile([P, icols], i16)
            nc.sync.dma_start(out=idx[:], in_=idx_d)

            F = cpool.tile([P, tot * HA], bf16)
            F3 = F.rearrange("p (n h) -> p n h", n=tot)

            ident = cpool.tile([P, P], bf16)
            nc.sync.dma_start(out=ident[:], in_=id_d)

            col = 0
            for ci, (s0, n) in enumerate(chunks):
                k = n // P  # slots covered
                g0 = s0 // P
                nc.gpsimd.dma_gather(
                    F3[:, g0 : g0 + k, :],
                    tab_d,
                    idx[:, col : col + n // 16],
                    n,
                    n,
                    HA,
                    queue_num=ci % N_QUEUES,
                )
                col += n // 16

            # chunk boundaries in global stream-slot space
            cb = sorted({s0 // P for s0, _ in chunks} | {tot})

            zsrc_t, zl_t, z2_t, e_t, deng_t, acc_t, gi_t = {}, {}, {}, {}, {}, {}, {}
            ngroups = [0] * S_TILES
            for (t, lo, hi, gpos) in blocks:
                splits = sorted({gpos, gpos + (hi - lo)} | {x for x in cb if gpos < x < gpos + (hi - lo)})
                ngroups[t] += len(splits) - 1

            for (t, lo, hi, gpos) in blocks:
                ncc = ncc_list[t]
                if lo == 0:
                    zsrc = spool.tile([P, 1], f32, name="zsrc")
                    nc.vector.tensor_copy(zsrc[:], F3[:, gpos, ZS_COL].unsqueeze(1))
                    zsrc_t[t] = zsrc
                    zl_t[t] = spool.tile([P, ncc], f32, name="zl")
                    z2_t[t] = spool.tile([P, ncc], f32, name="z2")
                    e_t[t] = spool.tile([P, ncc], f32, name="e")
                    deng_t[t] = spool.tile([P, ngroups[t]], f32, name="deng")
                    acc_t[t] = ppool.tile([P, H], f32, name="acc")
                    gi_t[t] = 0
                zsrc, zl, z2, e = zsrc_t[t], zl_t[t], z2_t[t], e_t[t]
                deng, acc = deng_t[t], acc_t[t]
                splits = sorted({gpos, gpos + (hi - lo)} | {x for x in cb if gpos < x < gpos + (hi - lo)})
                for ga, gb in zip(splits[:-1], splits[1:]):
                    la = lo + (ga - gpos)
                    lb = la + (gb - ga)
                    gi = gi_t[t]
                    gi_t[t] += 1
                    nc.vector.tensor_scalar_add(
                        z2[:, la:lb], F3[:, ga:gb, ZC_COL], zsrc[:]
                    )
                    nc.vector.scalar_tensor_tensor(
                        out=zl[:, la:lb],
                        in0=z2[:, la:lb],
                        scalar=SLOPE,
                        in1=z2[:, la:lb],
                        op0=Alu.mult,
                        op1=Alu.max,
                    )
                    nc.scalar.activation(
                        e[:, la:lb],
                        zl[:, la:lb],
                        Act.Exp,
                        accum_out=deng[:, gi : gi + 1],
                    )
                    for g in range(ga, gb):
                        ln = la + (g - ga)
                        if ln % DG_DVE_MOD < DG_DVE_TAKE:
                            dg = dpool.tile([P, P], bf16, name="dg")
                            nc.vector.tensor_scalar_mul(
                                dg[:], ident[:], e[:, ln : ln + 1]
                            )
                            lhsT, rhs = dg[:], F3[:, g, 0:H]
                        else:
                            fw = wpool.tile([P, H], bf16, name="fw")
                            nc.scalar.mul(fw[:], F3[:, g, 0:H], e[:, ln : ln + 1])
                            lhsT, rhs = ident[:], fw[:]
                        nc.tensor.matmul(
                            out=acc[:],
                            lhsT=lhsT,
                            rhs=rhs,
                            start=(ln == 0),
                            stop=(ln == ncc - 1),
                        )
                if hi == ncc:
                    rows = slice(t * P, (t + 1) * P)
                    den = spool.tile([P, 1], f32, name="den")
                    nc.vector.tensor_reduce(den[:], deng[:], axis=X, op=Alu.add)
                    rden = spool.tile([P, 1], f32, name="rden")
                    nc.vector.reciprocal(rden[:], den[:])
                    o = spool.tile([P, H], f32, name="o")
                    nc.scalar.mul(o[:], acc[:], rden[:])
                    nc.sync.dma_start(out=out_d[rows, :], in_=o[:])

    nc.compile()
    return nc


def _get_nc(ncc_list, u_pad):
    key = (tuple(ncc_list), u_pad)
    if key not in _CACHE:
        _CACHE[key] = _build_nc(key)
    return _CACHE[key]


def _ensure_axon_hooks():
    """Provide antenv.axon_hooks if the image lacks it, so trace=True /
    BASS_TRACE=1 profiling requests don't crash run_bass_kernel_spmd."""
    import sys
    import types

    try:
        import antenv.axon_hooks  # noqa: F401

        return
    except ImportError:
        pass
    try:
        import antenv
    except ImportError:
        return
    mod = types.ModuleType("antenv.axon_hooks")
    state = {"hook": None}

    def set_axon_ntff_profile_hook(h):
        state["hook"] = h

    def get_axon_ntff_profile_hook():
        if state["hook"] is None:
            try:
                from trn_agent_boot.trn_boot import _ntff_profile_via_ctypes

                state["hook"] = _ntff_profile_via_ctypes("/opt/axon/libaxon_pjrt.so")
            except Exception:
                return None
        return state["hook"]

    mod.set_axon_ntff_profile_hook = set_axon_ntff_profile_hook
    mod.get_axon_ntff_profile_hook = get_axon_ntff_profile_hook
    sys.modules["antenv.axon_hooks"] = mod
    antenv.axon_hooks = mod


def _prepare(inputs):
    """Host-side prep: per-core compact bf16 tables + slot-major int16
    index streams in the dma_gather wrapped layout."""
    node_ids = np.asarray(inputs["node_ids"]).astype(np.int64).reshape(B, S)
    neighs = np.asarray(inputs["neighs"]).astype(np.int64).reshape(B, S, N)
    mask = np.asarray(inputs["mask"]).astype(np.int32).reshape(B, S, N)
    emb = np.ascontiguousarray(np.asarray(inputs["emb_table"], dtype=np.float32))
    a_w = np.asarray(inputs["a_w"], dtype=np.float32).reshape(2 * H, 1)
    a_b = np.asarray(inputs["a_b"], dtype=np.float32)

    aws = a_w[:H, 0]
    awc = a_w[H:, 0]
    ab = np.float32(a_b.reshape(-1)[0])

    # Compact candidates: unmasked neighbors first, self at slot 0, pads
    # point at the sentinel. Sort nodes by unmasked count (desc) so later
    # tiles need fewer slots.
    un_cnt = (mask == 0).sum(axis=-1)  # [B, S]
    perm = np.argsort(-un_cnt, axis=1, kind="stable")
    nid_p = np.take_along_axis(node_ids, perm, axis=1)
    nbr_p = np.take_along_axis(neighs, perm[..., None], axis=1)
    msk_p = np.take_along_axis(mask, perm[..., None], axis=1)
    cnt_p = np.take_along_axis(un_cnt, perm, axis=1)

    cnt_t = cnt_p.reshape(B, S_TILES, P)
    ncc_list = [max(int(cnt_t[:, t, :].max()) + 1, 2) for t in range(S_TILES)]
    ncc = max(ncc_list)

    order = np.argsort(msk_p, axis=-1, kind="stable")
    sneighs = np.take_along_axis(nbr_p, order, axis=-1)
    cands = np.empty((B, S, ncc), np.int64)
    cands[..., 0] = nid_p
    cands[..., 1:] = sneighs[..., : ncc - 1]
    ks = np.arange(1, ncc)[None, None, :]
    cands[..., 1:][ks > cnt_p[..., None]] = SENT

    # Per-core compact tables and local-id index streams
    tabs, idx_streams, u_list = [], [], []
    for c in range(N_CORES):
        uniq, inv = np.unique(cands[c], return_inverse=True)
        u = len(uniq)
        assert u <= 32000, u
        loc = inv.reshape(S, ncc).astype(np.int16)
        tab = np.zeros((u, HA), dtype=ml_dtypes.bfloat16)
        real = uniq != SENT
        rows = emb[uniq[real]]
        tab[real, 0:H] = rows.astype(ml_dtypes.bfloat16)
        zc = rows @ awc + ab
        zs = rows @ aws
        np.clip(zc, -30.0, 30.0, out=zc)
        np.clip(zs, -30.0, 30.0, out=zs)
        tab[real, ZC_COL] = zc.astype(ml_dtypes.bfloat16)
        tab[real, ZS_COL] = zs.astype(ml_dtypes.bfloat16)
        tab[~real, ZC_COL] = np.float32(ZSENT)
        tabs.append(tab)
        u_list.append(u)

        # tile-interleaved, slot-major index stream
        stream = []
        for (t, lo, hi, gpos) in _stream_blocks(ncc_list):
            blk = loc[t * P : (t + 1) * P, lo:hi]  # [P, hi-lo]
            stream.append(blk.T.reshape(-1))  # slot-major
        idx_streams.append(np.concatenate(stream))

    u_pad = max(u_list)
    tabs = [
        np.ascontiguousarray(np.vstack([t, np.zeros((u_pad - len(t), HA), t.dtype)]))
        if len(t) < u_pad
        else np.ascontiguousarray(t)
        for t in tabs
    ]

    # wrapped idx layout per chunk: idx i -> partition i%16, col i//16,
    # replicated across the 8 Q7-core stripes
    n_idx = sum(ncc_list) * P
    idxw_all = []
    for c in range(N_CORES):
        st = idx_streams[c]
        assert len(st) == n_idx
        cols = []
        s = 0
        while s < n_idx:
            n = min(CHUNK_IDXS, n_idx - s)
            blk = st[s : s + n].reshape(n // 16, 16).T
            cols.append(np.tile(blk, (8, 1)))
            s += n
        idxw_all.append(np.ascontiguousarray(np.hstack(cols).astype(np.int16)))

    return tabs, idxw_all, perm, ncc_list, u_pad


def kernel(**inputs) -> np.ndarray:
    _ensure_axon_hooks()
    from concourse.bass_utils import run_bass_kernel_spmd

    tabs, idxw_all, perm, ncc_list, u_pad = _prepare(inputs)
    nc = _get_nc(ncc_list, u_pad)
    identity = np.ascontiguousarray(np.eye(P, dtype=ml_dtypes.bfloat16))
    in_maps = [
        {"tab": tabs[c], "idxs": idxw_all[c], "identity": identity}
        for c in range(N_CORES)
    ]
    core_ids = list(range(N_CORES))
    try:
        res = run_bass_kernel_spmd(nc, in_maps, core_ids=core_ids)
    except Exception:
        # transient device wedge — retry once
        res = run_bass_kernel_spmd(nc, in_maps, core_ids=core_ids)
    _CACHE["last_res"] = res
    out = np.empty((N_CORES, S, H), np.float32)
    for c in range(N_CORES):
        out[c, perm[c], :] = res.results[c]["out"]
    return out
